# revision 5
# baseline (speedup 1.0000x reference)
"""Trainium2 Bass kernel for nn_CrossAttention (B=4, NQ=NK=1024, C=1024, H=16).

Sharding (8 cores): 2-way batch x 4-way head split. Core c handles batches
[2*(c//4), 2*(c//4)+1] and heads [4*(c%4) .. 4*(c%4)+4). Each core computes
q/k/v projections for its head slice, full NQxNK attention for its 8 (b,h)
pairs, and a partial output projection; the host sums the 4 head-shard
partials per batch and adds bp.

Per-core pipeline:
  - projections in bf16 (host pre-casts inputs/weights, folds 1/sqrt(D) into Wq)
  - scores S^T[nk,nq] = k_T^T q_T in fp32r (full PE rate, ~1.5e-4 accuracy)
  - P = exp(S) * exp(bias)  (ACT exp from PSUM -> bf16, DVE mul with
    host-precomputed exp(attn_bias), both transposed to [nk,nq])
  - AV with ones-augmented V gives y and the softmax denominator in one
    accumulation chain; batched DVE reciprocal + DRAM-bounce broadcast
    normalizes y
  - output projection in fp32r, partials returned as fp16
"""

import os
import sys

if "/opt/trn_rl_repo" not in sys.path:
    sys.path.insert(0, "/opt/trn_rl_repo")

import numpy as np
import ml_dtypes

import concourse.bass as bass
import concourse.mybir as mybir
import concourse.tile as tile
from concourse import bacc, bass_utils

F32 = mybir.dt.float32
F32R = mybir.dt.float32r
F16 = mybir.dt.float16
BF16 = mybir.dt.bfloat16
AF = mybir.ActivationFunctionType
BF16NP = ml_dtypes.bfloat16

B, NQ, NK, C, H = 4, 1024, 1024, 1024, 16
D = C // H  # 64
NB = 2   # batches per core
NH = 4   # heads per core
HD = NH * D  # 256
N_CORES = 8

_compiled = None  # (nc, exec cache)
last_exec_time_ns = None


def _build():
    nc = bacc.Bacc("TRN2", debug=False)

    d_xq = nc.dram_tensor("xq_t", [NB, C, NQ], BF16, kind="ExternalInput").ap()
    d_xk = nc.dram_tensor("xk_t", [NB, C, NK], BF16, kind="ExternalInput").ap()
    d_wq = nc.dram_tensor("wq_t", [C, HD], BF16, kind="ExternalInput").ap()
    d_wk = nc.dram_tensor("wk_t", [C, HD], BF16, kind="ExternalInput").ap()
    d_wv = nc.dram_tensor("wv_t", [C, HD], BF16, kind="ExternalInput").ap()
    d_wp = nc.dram_tensor("wp_t", [HD, C], F32R, kind="ExternalInput").ap()
    d_eb = nc.dram_tensor("eb_t", [NH, NK, NQ], BF16, kind="ExternalInput").ap()
    d_bq = nc.dram_tensor("bq_s", [HD], F32, kind="ExternalInput").ap()
    d_bk = nc.dram_tensor("bk_s", [HD], F32, kind="ExternalInput").ap()
    d_bv = nc.dram_tensor("bv_r", [HD], BF16, kind="ExternalInput").ap()
    d_out = nc.dram_tensor("out_p", [NB, NQ, C], F16, kind="ExternalOutput").ap()
    d_recip = nc.dram_tensor("recip_scr", [NB, 8, 512], F32, kind="Internal").ap()

    with tile.TileContext(nc) as tc:
        with (
            tc.tile_pool(name="consts", bufs=1) as cp,
            tc.tile_pool(name="xs", bufs=2) as xp,
            tc.tile_pool(name="qk", bufs=1) as qkp,
            tc.tile_pool(name="vaug", bufs=20) as vp,
            tc.tile_pool(name="ebp", bufs=4) as ebp,
            tc.tile_pool(name="pp", bufs=3) as ppool,
            tc.tile_pool(name="y65p", bufs=16) as y65p,
            tc.tile_pool(name="ytp", bufs=2) as ytp,
            tc.tile_pool(name="small", bufs=2) as sp,
            tc.tile_pool(name="rbcp", bufs=3) as rbcp,
            tc.tile_pool(name="outp", bufs=3) as op,
            tc.tile_pool(name="spsum", bufs=2, space="PSUM") as s_pool,
            tc.tile_pool(name="ypsum", bufs=2, space="PSUM") as y_pool,
            tc.tile_pool(name="pjpsum", bufs=2, space="PSUM") as pj_pool,
        ):
            # ---- constants ----
            t_wq = cp.tile([128, 8, HD], BF16, tag="wq")
            nc.sync.dma_start(t_wq[:], d_wq.rearrange("(a p) o -> p a o", p=128))
            t_wk = cp.tile([128, 8, HD], BF16, tag="wk")
            nc.sync.dma_start(t_wk[:], d_wk.rearrange("(a p) o -> p a o", p=128))
            t_wv = cp.tile([128, 8, HD], BF16, tag="wv")
            nc.sync.dma_start(t_wv[:], d_wv.rearrange("(a p) o -> p a o", p=128))
            t_wp = cp.tile([128, 2, C], F32R, tag="wp")
            nc.sync.dma_start(t_wp[:], d_wp.rearrange("(a p) n -> p a n", p=128))
            t_bq = cp.tile([128, 2], F32, tag="bq")
            nc.sync.dma_start(t_bq[:], d_bq.rearrange("(a p) -> p a", p=128))
            t_bk = cp.tile([128, 2], F32, tag="bk")
            nc.sync.dma_start(t_bk[:], d_bk.rearrange("(a p) -> p a", p=128))
            t_bv = cp.tile([1, HD], BF16, tag="bv")
            nc.sync.dma_start(t_bv[:], d_bv.rearrange("(a o) -> a o", a=1))
            t_ones = cp.tile([1, 128], BF16, tag="ones")
            nc.vector.memset(t_ones[:], 1.0)

            # ---- staged per-batch state ----
            xq_t = [None] * NB
            xk_t = [None] * NB
            q_t = [None] * NB
            k_t = [None] * NB
            vaug = [[None] * 8 for _ in range(NB)]
            y65 = [[None] * 8 for _ in range(NB)]
            y_t = [None] * NB

            def load_x(b):
                xq_t[b] = xp.tile([128, 8, NQ], BF16, tag="xq", name=f"xq{b}")
                nc.sync.dma_start(
                    xq_t[b][:], d_xq[b].rearrange("(a p) n -> p a n", p=128)
                )
                xk_t[b] = xp.tile([128, 8, NK], BF16, tag="xk", name=f"xk{b}")
                nc.sync.dma_start(
                    xk_t[b][:], d_xk[b].rearrange("(a p) n -> p a n", p=128)
                )

            def proj(b):
                # q_T / k_T : [256(o), NQ] as [128, 2(oc), NQ] fp32
                q_t[b] = qkp.tile([128, 2, NQ], F32R, tag="qT", name=f"qT{b}")
                k_t[b] = qkp.tile([128, 2, NK], F32R, tag="kT", name=f"kT{b}")
                for dst, w_t, x_t, b_t in (
                    (q_t[b], t_wq, xq_t[b], t_bq),
                    (k_t[b], t_wk, xk_t[b], t_bk),
                ):
                    for oc in range(2):
                        for nqc in range(2):
                            ps = pj_pool.tile([128, 512], F32, tag="pj")
                            for cc in range(8):
                                nc.tensor.matmul(
                                    ps[:],
                                    w_t[:, cc, oc * 128:(oc + 1) * 128],
                                    x_t[:, cc, nqc * 512:(nqc + 1) * 512],
                                    start=(cc == 0),
                                    stop=(cc == 7),
                                )
                            nc.scalar.activation(
                                dst[:, oc, nqc * 512:(nqc + 1) * 512],
                                ps[:],
                                AF.Identity,
                                bias=b_t[:, oc:oc + 1],
                            )
                # v in [nk, hd] orientation, augmented with a ones column per head
                for nkc in range(8):
                    ps = pj_pool.tile([128, HD], F32, tag="pj")
                    for cc in range(8):
                        nc.tensor.matmul(
                            ps[:],
                            xk_t[b][:, cc, nkc * 128:(nkc + 1) * 128],
                            t_wv[:, cc, :],
                            start=(cc == 0),
                            stop=False,
                        )
                    nc.tensor.matmul(
                        ps[:], t_ones[:, 0:128], t_bv[:], start=False, stop=True
                    )
                    va = vp.tile([128, NH, D + 1], BF16, tag="vaug", name=f"va{b}_{nkc}")
                    nc.vector.memset(va[:, :, D:D + 1], 1.0)
                    nc.vector.tensor_copy(
                        va[:, :, 0:D], ps.rearrange("p (h d) -> p h d", h=NH)
                    )
                    vaug[b][nkc] = va

            def attn(b):
                for h in range(NH):
                    hp, hr = h // 2, (h % 2) * 64
                    y_ps = [
                        y_pool.tile([65, 512], F32, tag="y", name=f"y{b}_{h}_{i}")
                        for i in range(2)
                    ]
                    for kc in range(8):
                        s_ps = s_pool.tile([128, 1024], F32, tag="s")
                        for nqc in range(2):
                            nc.tensor.matmul(
                                s_ps[:, nqc * 512:(nqc + 1) * 512],
                                k_t[b][hr:hr + 64, hp, kc * 128:(kc + 1) * 128],
                                q_t[b][hr:hr + 64, hp, nqc * 512:(nqc + 1) * 512],
                                start=True,
                                stop=True,
                            )
                        p0 = ppool.tile([128, 1024], BF16, tag="p0")
                        nc.scalar.activation(p0[:], s_ps[:], AF.Exp)
                        ebt = ebp.tile([128, 1024], BF16, tag="eb")
                        nc.sync.dma_start(
                            ebt[:], d_eb[h, kc * 128:(kc + 1) * 128, :]
                        )
                        p = ppool.tile([128, 1024], BF16, tag="p")
                        nc.vector.tensor_mul(p[:], p0[:], ebt[:])
                        for nqc in range(2):
                            nc.tensor.matmul(
                                y_ps[nqc][0:65, :],
                                vaug[b][kc][:, h, :],
                                p[:, nqc * 512:(nqc + 1) * 512],
                                start=(kc == 0),
                                stop=(kc == 7),
                            )
                    for nqc in range(2):
                        t = y65p.tile([65, 512], F32, tag="y65", name=f"y65_{b}_{h}_{nqc}")
                        nc.vector.tensor_copy(t[:], y_ps[nqc][0:65, :])
                        y65[b][h * 2 + nqc] = t

            def normalize(b):
                den = sp.tile([8, 512], F32, tag="den")
                for idx in range(8):
                    nc.sync.dma_start(
                        den[idx:idx + 1, :], y65[b][idx][64:65, :]
                    )
                rec = sp.tile([8, 512], F32, tag="rec")
                nc.vector.reciprocal(rec[:], den[:])
                nc.sync.dma_start(d_recip[b], rec[:])
                y_t[b] = ytp.tile([128, 2, NQ], F32R, tag="yT", name=f"yT{b}")
                for h in range(NH):
                    for nqc in range(2):
                        idx = h * 2 + nqc
                        rbc = rbcp.tile([64, 512], F32, tag="rbc")
                        nc.sync.dma_start(
                            rbc[:],
                            d_recip[b:b + 1, idx:idx + 1, :]
                            .rearrange("a b n -> (a b) n")
                            .broadcast_to([64, 512]),
                        )
                        if h % 2 == 0:
                            nc.vector.tensor_mul(
                                y_t[b][0:64, h // 2, nqc * 512:(nqc + 1) * 512],
                                y65[b][idx][0:64, :],
                                rbc[:],
                            )
                        else:
                            ntmp = rbcp.tile([64, 512], F32R, tag="ntmp", bufs=2)
                            nc.vector.tensor_mul(
                                ntmp[:], y65[b][idx][0:64, :], rbc[:]
                            )
                            nc.sync.dma_start(
                                y_t[b][64:128, h // 2, nqc * 512:(nqc + 1) * 512],
                                ntmp[:],
                            )

            def outproj(b):
                for mq in range(8):
                    for ncc in range(2):
                        ps = pj_pool.tile([128, 512], F32, tag="pj")
                        for j in range(2):
                            nc.tensor.matmul(
                                ps[:],
                                y_t[b][:, j, mq * 128:(mq + 1) * 128],
                                t_wp[:, j, ncc * 512:(ncc + 1) * 512],
                                start=(j == 0),
                                stop=(j == 1),
                            )
                        ot = op.tile([128, 512], F16, tag="out")
                        nc.vector.tensor_copy(ot[:], ps[:])
                        nc.sync.dma_start(
                            d_out[b, mq * 128:(mq + 1) * 128,
                                  ncc * 512:(ncc + 1) * 512],
                            ot[:],
                        )

            load_x(0)
            proj(0)
            load_x(1)
            attn(0)
            proj(1)
            normalize(0)
            outproj(0)
            attn(1)
            normalize(1)
            outproj(1)

    nc.finalize()
    return nc


def kernel(**inputs):
    global _compiled, last_exec_time_ns
    query = np.asarray(inputs["query"], np.float32)
    key = np.asarray(inputs["key"], np.float32)
    attn_bias = np.asarray(inputs["attn_bias"], np.float32)
    Wq = np.asarray(inputs["Wq"], np.float32)
    bq = np.asarray(inputs["bq"], np.float32)
    Wk = np.asarray(inputs["Wk"], np.float32)
    bk = np.asarray(inputs["bk"], np.float32)
    Wv = np.asarray(inputs["Wv"], np.float32)
    bv = np.asarray(inputs["bv"], np.float32)
    Wp = np.asarray(inputs["Wp"], np.float32)
    bp = np.asarray(inputs["bp"], np.float32)

    scale = 1.0 / np.sqrt(D)

    # full-batch transposes / casts shared by cores
    xq_t_all = np.ascontiguousarray(query.transpose(0, 2, 1)).astype(BF16NP)
    xk_t_all = np.ascontiguousarray(key.transpose(0, 2, 1)).astype(BF16NP)
    eb_all = np.exp(attn_bias[0]).transpose(0, 2, 1)  # [H, NK, NQ] f32

    in_maps = []
    for c in range(N_CORES):
        bg, hq = c // 4, c % 4
        sl = slice(hq * HD, (hq + 1) * HD)
        in_maps.append({
            "xq_t": xq_t_all[2 * bg:2 * bg + 2],
            "xk_t": xk_t_all[2 * bg:2 * bg + 2],
            "wq_t": np.ascontiguousarray((Wq[sl, :] * scale).T).astype(BF16NP),
            "wk_t": np.ascontiguousarray(Wk[sl, :].T).astype(BF16NP),
            "wv_t": np.ascontiguousarray(Wv[sl, :].T).astype(BF16NP),
            "wp_t": np.ascontiguousarray(Wp[:, sl].T).astype(np.float32),
            "eb_t": np.ascontiguousarray(eb_all[4 * hq:4 * hq + 4]).astype(BF16NP),
            "bq_s": (bq[sl] * scale).astype(np.float32),
            "bk_s": bk[sl].astype(np.float32),
            "bv_r": bv[sl].astype(BF16NP),
        })

    if _compiled is None:
        _compiled = _build()
    nc = _compiled

    trace = bool(os.environ.get("KERNEL_TRACE"))
    res = bass_utils.run_bass_kernel_spmd(
        nc, in_maps, core_ids=list(range(N_CORES)), trace=trace
    )
    last_exec_time_ns = res.exec_time_ns

    out = np.zeros((B, NQ, C), np.float32)
    for c in range(N_CORES):
        bg = c // 4
        out[2 * bg:2 * bg + 2] += res.results[c]["out_p"].astype(np.float32)
    out += bp
    return out
